# revision 1
# baseline (speedup 1.0000x reference)
"""Multi-head attention (B=4, S=2048, D=1024, H=16, Dh=64) on 8 trn2 NeuronCores.

Sharding: core c -> (batch b = c//2, head-group g = c%2 of 8 heads).
Each core computes q/k/v projections for its 8 heads and the full attention,
writing o[b, :, 512*g : 512*(g+1)].  No collectives needed: the output's
feature dim is just the concatenation of per-head outputs.

Layout strategy (per core):
  - Host pre-transposes X (seq-major -> D-major) so the contraction dim D
    lands on SBUF partitions without on-chip transposes, and casts to bf16.
  - Projections compute qT/kT in [dh, seq] orientation (lhsT = W k-tile,
    rhs = X.T k-tile) and v in natural [seq, dh] orientation.
  - Scores are computed TRANSPOSED (scoresT[sk, sq] = kT.T @ qT) so that the
    exp'd attention matrix A.T is already in the [sk-partition, sq-free]
    layout the PV matmul needs as its stationary operand -> no transposes.
  - Softmax denominators come for free from a mask column appended to V
    (o_psum column 64 = sum over valid sk of A.T), so no reductions.
  - k-masking: V rows beyond V_len are zeroed on host and the mask column is
    0 there, so invalid sk contribute nothing to numerator or denominator.
    exp is computed without max-subtraction (scores are O(+-10), safe in
    fp32) which matches softmax exactly up to rounding.
  - q-masking + normalization fused: out_tile = o_psum[:, :64] *
    (maskq / sum) as a per-partition scalar multiply.

The program is compiled for SQT/SKT = ceil(max(Q_len)/128), ceil(max(V_len)/128)
tiles (shared SPMD program across the 8 cores), so work scales with the
actual sequence lengths.  Per-core smaller lengths are handled by the masks.
"""

import math

import numpy as np
import ml_dtypes


def _ensure_paths():
    import sys
    try:
        import concourse  # noqa: F401
        return
    except ImportError:
        pass
    for p in ("/opt/trn_rl_repo", "/root/.axon_site/_ro/trn_rl_repo"):
        if p not in sys.path:
            sys.path.insert(0, p)
    import concourse  # noqa: F401


P = 128          # SBUF partitions
D = 1024         # model dim
DH = 64          # head dim
HL = 8           # heads per core
E = HL * DH      # per-core output feature width (512)
NCORES = 8

_PROG_CACHE = {}

# exposed for test.py profiling reruns
_last_nc = None
_last_in_maps = None


def _build_program(SQT, SKT):
    """Build + bacc-compile the shared SPMD program for given tile counts."""
    _ensure_paths()
    import concourse.bass as bass  # noqa: F401
    import concourse.tile as tile
    from concourse import bacc, mybir

    BF = mybir.dt.bfloat16
    F32 = mybir.dt.float32
    Exp = mybir.ActivationFunctionType.Exp

    SQ = SQT * P
    SK = SKT * P
    QC = math.ceil(SQ / 512)   # sq chunks for matmul free dim / psum banks
    KC = math.ceil(SK / 512)
    KT = D // P                # 8 contraction tiles

    nc = bacc.Bacc("TRN2", target_bir_lowering=False, debug=False,
                   num_devices=NCORES)

    xqt = nc.dram_tensor("xqt", [D, SQ], BF, kind="ExternalInput").ap()
    xkt = nc.dram_tensor("xkt", [D, SK], BF, kind="ExternalInput").ap()
    xvt = nc.dram_tensor("xvt", [D, SK], BF, kind="ExternalInput").ap()
    wq = nc.dram_tensor("wq", [D, E], BF, kind="ExternalInput").ap()
    wk = nc.dram_tensor("wk", [D, E], BF, kind="ExternalInput").ap()
    wv = nc.dram_tensor("wv", [D, E], BF, kind="ExternalInput").ap()
    maskq = nc.dram_tensor("maskq", [SQT, P], F32, kind="ExternalInput").ap()
    maskk8 = nc.dram_tensor("maskk8", [SK, HL, 1], BF, kind="ExternalInput").ap()
    identd = nc.dram_tensor("ident", [P, P], F32, kind="ExternalInput").ap()
    out = nc.dram_tensor("out", [2048, E], F32, kind="ExternalOutput").ap()

    xqt_r = xqt.rearrange("(k p) s -> p k s", p=P)
    xkt_r = xkt.rearrange("(k p) s -> p k s", p=P)
    xvt_r = xvt.rearrange("(k p) s -> p k s", p=P)

    # at (A.T) buffering: double-buffer if it fits in SBUF alongside the rest
    at_bytes = SKT * min(SQ, 1024) * 2
    fixed_bytes = (3 * 8 * E * 2        # weights
                   + 3 * 8 * 512 * 2    # x stream bufs (proj phase)
                   + 4 * SQ * 2 + 4 * SK * 2 + SKT * HL * 65 * 2  # qt/kt/v
                   + 2 * SQ * 4         # oT bufs
                   + 4096)              # misc
    at_bufs = 2 if fixed_bytes + 2 * at_bytes < 188 * 1024 else 1

    with tile.TileContext(nc) as tc:
        with tc.tile_pool(name="const", bufs=1) as const, \
             tc.tile_pool(name="persist", bufs=1) as persist, \
             tc.tile_pool(name="atp", bufs=at_bufs) as atp, \
             tc.tile_pool(name="otp", bufs=2) as otp, \
             tc.tile_pool(name="small", bufs=6) as small:

            wq_sb = const.tile([P, KT, E], BF, tag="wq")
            wk_sb = const.tile([P, KT, E], BF, tag="wk")
            wv_sb = const.tile([P, KT, E], BF, tag="wv")
            maskq_sb = const.tile([P, SQT], F32, tag="mq")
            ident = const.tile([P, P], F32, tag="ident")
            nc.sync.dma_start(out=wq_sb, in_=wq.rearrange("(k p) e -> p k e", p=P))
            nc.sync.dma_start(out=wk_sb, in_=wk.rearrange("(k p) e -> p k e", p=P))
            nc.sync.dma_start(out=wv_sb, in_=wv.rearrange("(k p) e -> p k e", p=P))
            nc.sync.dma_start(out=maskq_sb, in_=maskq.rearrange("t p -> p t"))
            nc.sync.dma_start(out=ident, in_=identd)

            qt_sb = persist.tile([P, 4, SQ], BF, tag="qt")
            kt_sb = persist.tile([P, 4, SK], BF, tag="kt")
            # per-head qT with the unused partition half zeroed, so QK can use
            # full [128,128] lhsT tiles (the HAM activity monitor appears to
            # ignore partial-array matmuls, leaving the PE clock throttled)
            qt_pad = persist.tile([P, HL, SQ], BF, tag="qtp")
            # V tiles padded at the tail so lhsT can be read [128,128] wide
            v_sb = persist.tile([P, SKT, HL * (DH + 1) + DH - 1], BF, tag="v")
            nc.vector.memset(qt_pad, 0.0)
            # tail pad of v_sb is read as lhsT columns for head 7; zero it so
            # uninitialized SBUF (possibly NaN bit patterns) never reaches PSUM
            nc.vector.memset(v_sb[:, :, HL * (DH + 1):], 0.0)

            # ---- projections (own pools so SBUF/PSUM are released after) ----
            with tc.tile_pool(name="xs", bufs=3) as xs, \
                 tc.tile_pool(name="psp", bufs=2, space="PSUM") as psp:
                # qT, kT in [dh, seq] orientation, head-pair-major
                for dst, x_r, w_sb, nchunks, stot in (
                    (qt_sb, xqt_r, wq_sb, QC, SQ),
                    (kt_sb, xkt_r, wk_sb, KC, SK),
                ):
                    for c in range(nchunks):
                        c0 = 512 * c
                        ncols = min(512, stot - c0)
                        xt = xs.tile([P, KT, 512], BF, tag="x")
                        nc.sync.dma_start(out=xt[:, :, :ncols],
                                          in_=x_r[:, :, c0:c0 + ncols])
                        for p4 in range(4):
                            ps = psp.tile([P, 512], F32, tag="proj")
                            for k in range(KT):
                                nc.tensor.matmul(
                                    ps[:, :ncols],
                                    w_sb[:, k, P * p4:P * (p4 + 1)],
                                    xt[:, k, :ncols],
                                    start=(k == 0), stop=(k == KT - 1))
                            nc.vector.tensor_copy(out=dst[:, p4, c0:c0 + ncols],
                                                  in_=ps[:, :ncols])

                # v in natural [seq, dh] orientation + mask column
                for m in range(SKT):
                    xt = xs.tile([P, KT, 512], BF, tag="x")
                    nc.sync.dma_start(out=xt[:, :, :P],
                                      in_=xvt_r[:, :, P * m:P * (m + 1)])
                    ps = psp.tile([P, 512], F32, tag="proj")
                    for k in range(KT):
                        nc.tensor.matmul(ps, xt[:, k, :P], wv_sb[:, k, :],
                                         start=(k == 0), stop=(k == KT - 1))
                    v_m = v_sb[:, m, 0:HL * (DH + 1)].rearrange(
                        "p (h c) -> p h c", c=DH + 1)
                    nc.vector.tensor_copy(
                        out=v_m[:, :, 0:DH],
                        in_=ps.rearrange("p (h d) -> p h d", h=HL))
                    nc.sync.dma_start(out=v_m[:, :, DH:DH + 1],
                                      in_=maskk8[P * m:P * (m + 1)])

                # scatter qT halves into per-head zero-padded buffers
                # (partition-aligned SBUF->SBUF copies; even heads occupy
                # partitions 0:64 matching their kT rows, odd heads 64:128)
                for p4 in range(4):
                    nc.sync.dma_start(out=qt_pad[0:DH, 2 * p4, :],
                                      in_=qt_sb[0:DH, p4, :])
                    nc.sync.dma_start(out=qt_pad[DH:P, 2 * p4 + 1, :],
                                      in_=qt_sb[DH:P, p4, :])

            # ---- attention: software-pipelined, sq split into passes ----
            # The PE executes in program order; with a single score-psum the
            # chain QK(t) -> exp(t) -> QK(t+1) serializes PE behind ACT and
            # the HAM clock gate never warms.  Splitting sq into <=1024-wide
            # passes makes the score psum 2 banks, so it double-buffers
            # (bufs=2), and PV/transpose work of the previous (head, pass) is
            # emitted before each QK step to keep the in-order PE stream dense.
            chunk_list = []
            off = 0
            while off < SQ:
                n = min(512, SQ - off)
                chunk_list.append((off, n))
                off += n
            passes = []  # (pass_off, [chunk sizes]) with total <= 1024
            cur = []
            cur_off = 0
            for (co, n) in chunk_list:
                if sum(cur) + n > 1024 and cur:
                    passes.append((cur_off, cur))
                    cur = []
                    cur_off = co
                cur.append(n)
            passes.append((cur_off, cur))

            with tc.tile_pool(name="psq", bufs=2, space="PSUM") as psq, \
                 tc.tile_pool(name="psot", bufs=2, space="PSUM") as psot, \
                 tc.tile_pool(name="pstr", bufs=2, space="PSUM") as pstr:

              def emit_qk_step(h, t, at, poff, csizes, psz):
                  p4 = h // 2
                  ps = psq.tile([P, 1024], F32, tag="qk",
                                name=f"qk_{h}_{t}_{poff}")
                  c0 = 0
                  for n in csizes:
                      nc.tensor.matmul(
                          ps[:, c0:c0 + n],
                          kt_sb[:, p4, P * t:P * (t + 1)],
                          qt_pad[:, h, poff + c0:poff + c0 + n],
                          start=True, stop=True)
                      c0 += n
                  nc.scalar.activation(out=at[:, t, :psz], in_=ps[:, :psz],
                                       func=Exp, scale=0.125)

              def pv_pieces(h, at, poff, csizes, psz):
                  """Closures emitting PV + transpose + normalize for one
                  (head, pass)."""
                  ot = otp.tile([P, 1024], F32, tag="ot",
                                name=f"ot_{h}_{poff}")
                  pieces = []

                  def mk_pv(c0, n, t0, t1, po_box):
                      def go():
                          if t0 == 0:
                              po_box[0] = psot.tile([P, 512], F32, tag="o",
                                                    name=f"po_{h}_{poff}_{c0}")
                          po = po_box[0]
                          h0 = h * (DH + 1)
                          for t in range(t0, t1):
                              nc.tensor.matmul(po[:, :n],
                                               v_sb[:, t, h0:h0 + P],
                                               at[:, t, c0:c0 + n],
                                               start=(t == 0),
                                               stop=(t == SKT - 1))
                          if t1 == SKT:
                              nc.vector.tensor_copy(out=ot[:, c0:c0 + n],
                                                    in_=po[:, :n])
                      return go

                  c0 = 0
                  for n in csizes:
                      box = [None]
                      if SKT > 6:
                          half_t = (SKT + 1) // 2
                          pieces.append(mk_pv(c0, n, 0, half_t, box))
                          pieces.append(mk_pv(c0, n, half_t, SKT, box))
                      else:
                          pieces.append(mk_pv(c0, n, 0, SKT, box))
                      c0 += n

                  def mk_tr(lsq, gsq):
                      def go():
                          po2 = pstr.tile([P, P], F32, tag="tr",
                                          name=f"po2_{h}_{gsq}")
                          # transpose as a REGULAR full-array fp32 matmul
                          # (ot.T @ I); identity rows >= 65 are zero so the
                          # junk rows of ot never reach the output
                          nc.tensor.matmul(po2, ot[:, P * lsq:P * (lsq + 1)],
                                           ident, start=True, stop=True)
                          rc = small.tile([P, 1], F32, tag="rc",
                                          name=f"rc_{h}_{gsq}")
                          sc = small.tile([P, 1], F32, tag="sc",
                                          name=f"sc_{h}_{gsq}")
                          nc.vector.reciprocal(rc, po2[:, DH:DH + 1])
                          nc.vector.tensor_mul(sc, rc, maskq_sb[:, gsq:gsq + 1])
                          ob = small.tile([P, DH], F32, tag="ob",
                                          name=f"ob_{h}_{gsq}")
                          nc.vector.tensor_scalar_mul(ob, po2[:, 0:DH], sc)
                          nc.sync.dma_start(
                              out=out[P * gsq:P * (gsq + 1),
                                      DH * h:DH * (h + 1)],
                              in_=ob)
                      return go

                  for lsq in range(psz // P):
                      pieces.append(mk_tr(lsq, poff // P + lsq))
                  return pieces

              prev_pieces = []
              for poff, csizes in passes:
                  psz = sum(csizes)
                  for h in range(HL):
                      at = atp.tile([P, SKT, psz], BF, tag="at",
                                    name=f"at_{h}_{poff}")
                      L = len(prev_pieces)
                      done = 0
                      for t in range(SKT):
                          upto = (L * (t + 1)) // SKT
                          while done < upto:
                              prev_pieces[done]()
                              done += 1
                          emit_qk_step(h, t, at, poff, csizes, psz)
                      while done < L:
                          prev_pieces[done]()
                          done += 1
                      prev_pieces = pv_pieces(h, at, poff, csizes, psz)
              for piece in prev_pieces:
                  piece()

    nc.compile()
    return nc


def _get_program(SQT, SKT):
    key = (SQT, SKT)
    if key not in _PROG_CACHE:
        _PROG_CACHE[key] = _build_program(SQT, SKT)
    return _PROG_CACHE[key]


def _ident128():
    i = np.zeros((P, P), dtype=np.float32)
    for k in range(DH + 1):
        i[k, k] = 1.0
    return i


def kernel(Q_seq, K_seq, V_seq, WQ, WK, WV, Q_len, V_len):
    global _last_nc, _last_in_maps
    _ensure_paths()
    from concourse.bass_utils import run_bass_kernel_spmd

    Q_seq = np.asarray(Q_seq, dtype=np.float32)
    K_seq = np.asarray(K_seq, dtype=np.float32)
    V_seq = np.asarray(V_seq, dtype=np.float32)
    WQ = np.asarray(WQ, dtype=np.float32)
    WK = np.asarray(WK, dtype=np.float32)
    WV = np.asarray(WV, dtype=np.float32)
    Q_len = np.asarray(Q_len).reshape(-1)
    V_len = np.asarray(V_len).reshape(-1)

    B, S, _ = Q_seq.shape
    BF = ml_dtypes.bfloat16

    SQT = max(1, math.ceil(int(Q_len.max()) / P))
    SKT = max(1, math.ceil(int(V_len.max()) / P))
    SQ, SK = SQT * P, SKT * P

    nc = _get_program(SQT, SKT)

    in_maps = []
    for c in range(NCORES):
        b, g = c // 2, c % 2
        ql, vl = int(Q_len[b]), int(V_len[b])
        mk = (np.arange(SK) < vl)
        xq = np.ascontiguousarray(Q_seq[b, :SQ].T).astype(BF)
        xk = np.ascontiguousarray(K_seq[b, :SK].T).astype(BF)
        xv = np.ascontiguousarray((V_seq[b, :SK] * mk[:, None]).T).astype(BF)
        in_maps.append({
            "xqt": xq,
            "xkt": xk,
            "xvt": xv,
            "wq": np.ascontiguousarray(WQ[:, E * g:E * (g + 1)]).astype(BF),
            "wk": np.ascontiguousarray(WK[:, E * g:E * (g + 1)]).astype(BF),
            "wv": np.ascontiguousarray(WV[:, E * g:E * (g + 1)]).astype(BF),
            "maskq": (np.arange(SQ) < ql).astype(np.float32).reshape(SQT, P),
            "maskk8": np.repeat(mk.astype(BF)[:, None], HL, axis=1)[..., None],
            "ident": _ident128(),
        })

    res = run_bass_kernel_spmd(nc, in_maps, core_ids=list(range(NCORES)))
    _last_nc, _last_in_maps = nc, in_maps

    full = np.zeros((B, S, 2 * E), dtype=np.float32)
    for c in range(NCORES):
        b, g = c // 2, c % 2
        o = res.results[c]["out"]
        # rows >= SQ are never written by the kernel; keep host zeros there
        full[b, :SQ, E * g:E * (g + 1)] = o[:SQ]
    return full



# revision 4
# speedup vs baseline: 1.4365x; 1.4365x over previous
"""Multi-head attention (B=4, S=2048, D=1024, H=16, Dh=64) on 8 trn2 NeuronCores.

Load-balanced sharding: per-batch sequence lengths differ wildly
(Q_len/V_len up to 2048, down to a few hundred), so a uniform
(batch, head-group) split leaves most cores idle-padded to the largest
batch.  Instead, batches are sorted by attention cost (SQT*SKT tiles) and:

  cores 0-3 -> biggest batch, 4 heads each   (arm 0)
  cores 4-5 -> 2nd batch,     8 heads each   (arm 1)
  core  6   -> 3rd batch,    16 heads        (arm 2)
  core  7   -> 4th batch,    16 heads        (arm 2)

The three shapes run as arms of a tc.Switch dispatched on a per-core
`vid` input, so a single SPMD program gives every core a program sized
to its own workload.  If the lengths happen to be balanced (cost of this
split >= the uniform split), a single-arm uniform program is used.

Per-arm kernel (NH heads, SQT/SKT 128-row tiles):
  - Host pre-transposes X (seq-major -> D-major) and packs this core's
    NH heads' weight columns; everything bf16.
  - Projections: qT/kT in [dh, seq] orientation (head pairs stacked on
    partitions), v in natural [seq, dh] orientation with a ones column
    appended per head (softmax denominator comes out of the PV matmul).
  - Scores computed TRANSPOSED (scoresT[sk, sq] = kT.T @ qT) via
    zero-padded per-head qT so QK uses full [128,128] lhsT tiles.
  - exp on the scalar engine (scale=1/8, no max-subtraction: scores are
    O(+-10), exact in fp32).
  - PV: po[128q, 65] += at[:, t, qblock].T @ v[:, t, head] accumulated
    over k-tiles -- output lands directly in [q, feature] orientation
    (no transposes), denominator in column 64.
  - normalize+q-mask on vector engine: out = po[:, :64] * (maskq/po[:,64]).
  - k-masking: V rows beyond V_len zeroed on host, ones-column zero
    there, so invalid sk contribute nothing.
"""

import math

import numpy as np
import ml_dtypes


def _ensure_paths():
    import sys
    try:
        import concourse  # noqa: F401
        return
    except ImportError:
        pass
    for p in ("/opt/trn_rl_repo", "/root/.axon_site/_ro/trn_rl_repo"):
        if p not in sys.path:
            sys.path.insert(0, p)
    import concourse  # noqa: F401


P = 128          # SBUF partitions
D = 1024         # model dim
DH = 64          # head dim
H = 16           # total heads
KT = D // P      # contraction tiles for projections
NCORES = 8

_PROG_CACHE = {}

# exposed for test.py profiling reruns
_last_nc = None
_last_in_maps = None


def _plan(Q_len, V_len):
    """Return (arm_specs, assignment, dims) for the given lengths.

    arm_specs: tuple of (NH, SQT, SKT) per arm.
    assignment: per core (arm, batch, head_lo).
    """
    SQT = [max(1, math.ceil(int(q) / P)) for q in Q_len]
    SKT = [max(1, math.ceil(int(v) / P)) for v in V_len]
    order = sorted(range(4), key=lambda b: -SQT[b] * SKT[b])
    o0, o1, o2, o3 = order
    arm_specs = (
        (4, SQT[o0], SKT[o0]),
        (8, SQT[o1], SKT[o1]),
        (16, max(SQT[o2], SQT[o3]), max(SKT[o2], SKT[o3])),
    )
    assignment = (
        [(0, o0, 4 * c) for c in range(4)]
        + [(1, o1, 8 * c) for c in range(2)]
        + [(2, o2, 0), (2, o3, 0)]
    )
    cost_new = max(nh * sq * sk for nh, sq, sk in arm_specs)
    cost_uniform = 8 * max(SQT) * max(SKT)
    if cost_new >= cost_uniform:
        arm_specs = ((8, max(SQT), max(SKT)),)
        assignment = [(0, c // 2, 8 * (c % 2)) for c in range(NCORES)]
    return arm_specs, assignment


def _passes(SQ):
    """Split SQ into 512-chunks grouped into <=1024-wide passes."""
    chunks = []
    off = 0
    while off < SQ:
        n = min(512, SQ - off)
        chunks.append((off, n))
        off += n
    passes, cur, cur_off = [], [], 0
    for (co, n) in chunks:
        if sum(cur) + n > 1024 and cur:
            passes.append((cur_off, cur))
            cur = []
            cur_off = co
        cur.append(n)
    passes.append((cur_off, cur))
    return passes


def _emit_arm(nc, tc, tensors, arm, NH, SQT, SKT):
    """Emit one arm's full pipeline (projections + attention)."""
    import concourse.tile as tile  # noqa: F401
    from concourse import mybir

    BF = mybir.dt.bfloat16
    F32 = mybir.dt.float32
    Exp = mybir.ActivationFunctionType.Exp

    xqt, xkt, xvt, wq, wk, wv, maskq, maskk, out = tensors

    E = NH * DH
    NPAIR = NH // 2
    SQ, SK = SQT * P, SKT * P
    a = f"a{arm}"

    xqt_r = xqt.rearrange("(k p) s -> p k s", p=P)
    xkt_r = xkt.rearrange("(k p) s -> p k s", p=P)
    xvt_r = xvt.rearrange("(k p) s -> p k s", p=P)

    with tc.tile_pool(name=f"{a}_const", bufs=1) as const, \
         tc.tile_pool(name=f"{a}_persist", bufs=1) as persist:

        wq_sb = const.tile([P, KT, E], BF, tag="wq", name=f"{a}_wq")
        wk_sb = const.tile([P, KT, E], BF, tag="wk", name=f"{a}_wk")
        wv_sb = const.tile([P, KT, E], BF, tag="wv", name=f"{a}_wv")
        maskq_sb = const.tile([P, SQT], F32, tag="mq", name=f"{a}_mq")
        nc.sync.dma_start(out=wq_sb,
                          in_=wq.rearrange("(k p) e -> p k e", p=P)[:, :, 0:E])
        nc.sync.dma_start(out=wk_sb,
                          in_=wk.rearrange("(k p) e -> p k e", p=P)[:, :, 0:E])
        nc.sync.dma_start(out=wv_sb,
                          in_=wv.rearrange("(k p) e -> p k e", p=P)[:, :, 0:E])
        nc.sync.dma_start(out=maskq_sb,
                          in_=maskq.rearrange("t p -> p t")[:, 0:SQT])

        qt_sb = persist.tile([P, NPAIR, SQ], BF, tag="qt", name=f"{a}_qt")
        kt_sb = persist.tile([P, NPAIR, SK], BF, tag="kt", name=f"{a}_kt")
        # per-head qT with the unused partition half zeroed, so QK can use
        # full [128,128] lhsT tiles (partial-array matmuls leave the PE
        # clock throttled)
        qt_pad = persist.tile([P, NH, SQ], BF, tag="qtp", name=f"{a}_qtp")
        v_sb = persist.tile([P, SKT, NH * (DH + 1)], BF, tag="v",
                            name=f"{a}_v")
        nc.vector.memset(qt_pad, 0.0)

        QC = math.ceil(SQ / 512)
        KC = math.ceil(SK / 512)

        # ---- projections (own pools so SBUF/PSUM are released after) ----
        with tc.tile_pool(name=f"{a}_xs", bufs=3) as xs, \
             tc.tile_pool(name=f"{a}_psp", bufs=2, space="PSUM") as psp:
            for dst, x_r, w_sb, nchunks, stot in (
                (qt_sb, xqt_r, wq_sb, QC, SQ),
                (kt_sb, xkt_r, wk_sb, KC, SK),
            ):
                for c in range(nchunks):
                    c0 = 512 * c
                    ncols = min(512, stot - c0)
                    xt = xs.tile([P, KT, 512], BF, tag="x",
                                 name=f"{a}_x_{dst.name}_{c}")
                    nc.sync.dma_start(out=xt[:, :, :ncols],
                                      in_=x_r[:, :, c0:c0 + ncols])
                    for p4 in range(E // P):
                        ps = psp.tile([P, 512], F32, tag="proj",
                                      name=f"{a}_pp_{dst.name}_{c}_{p4}")
                        for k in range(KT):
                            nc.tensor.matmul(
                                ps[:, :ncols],
                                w_sb[:, k, P * p4:P * (p4 + 1)],
                                xt[:, k, :ncols],
                                start=(k == 0), stop=(k == KT - 1))
                        nc.vector.tensor_copy(out=dst[:, p4, c0:c0 + ncols],
                                              in_=ps[:, :ncols])

            # v in natural [seq, dh] orientation + ones column per head
            for m in range(SKT):
                xt = xs.tile([P, KT, 512], BF, tag="x", name=f"{a}_xv_{m}")
                nc.sync.dma_start(out=xt[:, :, :P],
                                  in_=xvt_r[:, :, P * m:P * (m + 1)])
                v_m = v_sb[:, m, :].rearrange("p (h c) -> p h c", c=DH + 1)
                for e0 in range(0, E, 512):
                    ew = min(512, E - e0)
                    ps = psp.tile([P, 512], F32, tag="proj",
                                  name=f"{a}_pv_{m}_{e0}")
                    for k in range(KT):
                        nc.tensor.matmul(ps[:, :ew], xt[:, k, :P],
                                         wv_sb[:, k, e0:e0 + ew],
                                         start=(k == 0), stop=(k == KT - 1))
                    h0 = e0 // DH
                    nc.vector.tensor_copy(
                        out=v_m[:, h0:h0 + ew // DH, 0:DH],
                        in_=ps[:, :ew].rearrange("p (h d) -> p h d", d=DH))
                nc.sync.dma_start(out=v_m[:, :, DH:DH + 1],
                                  in_=maskk[P * m:P * (m + 1), 0:NH])

            # scatter qT halves into per-head zero-padded buffers (even
            # heads occupy partitions 0:64 matching their kT rows, odd
            # heads 64:128)
            for pr in range(NPAIR):
                nc.sync.dma_start(out=qt_pad[0:DH, 2 * pr, :],
                                  in_=qt_sb[0:DH, pr, :])
                nc.sync.dma_start(out=qt_pad[DH:P, 2 * pr + 1, :],
                                  in_=qt_sb[DH:P, pr, :])

        # ---- attention: QK -> exp -> PV, software-pipelined per head ----
        with tc.tile_pool(name=f"{a}_atp", bufs=2) as atp, \
             tc.tile_pool(name=f"{a}_psq", bufs=2, space="PSUM") as psq, \
             tc.tile_pool(name=f"{a}_pop", bufs=4, space="PSUM") as pop, \
             tc.tile_pool(name=f"{a}_small", bufs=8) as small:

            def pv_pieces(h, at, poff, psz):
                pieces = []

                def mk(qb, gq):
                    def go():
                        po = pop.tile([P, DH + 1], F32, tag="po",
                                      name=f"{a}_po_{h}_{gq}")
                        for t in range(SKT):
                            nc.tensor.matmul(
                                po,
                                at[:, t, P * qb:P * (qb + 1)],
                                v_sb[:, t, (DH + 1) * h:(DH + 1) * (h + 1)],
                                start=(t == 0), stop=(t == SKT - 1))
                        rc = small.tile([P, 1], F32, tag="rc",
                                        name=f"{a}_rc_{h}_{gq}")
                        sc = small.tile([P, 1], F32, tag="sc",
                                        name=f"{a}_sc_{h}_{gq}")
                        nc.vector.reciprocal(rc, po[:, DH:DH + 1])
                        nc.vector.tensor_mul(sc, rc, maskq_sb[:, gq:gq + 1])
                        ob = small.tile([P, DH], F32, tag="ob",
                                        name=f"{a}_ob_{h}_{gq}")
                        nc.vector.tensor_scalar_mul(ob, po[:, 0:DH], sc)
                        nc.sync.dma_start(
                            out=out[P * gq:P * (gq + 1),
                                    DH * h:DH * (h + 1)],
                            in_=ob)
                    return go

                for qb in range(psz // P):
                    pieces.append(mk(qb, poff // P + qb))
                return pieces

            prev_pieces = []
            for poff, csizes in _passes(SQ):
                psz = sum(csizes)
                for h in range(NH):
                    at = atp.tile([P, SKT, psz], BF, tag="at",
                                  name=f"{a}_at_{h}_{poff}")
                    L = len(prev_pieces)
                    done = 0
                    for t in range(SKT):
                        upto = (L * (t + 1)) // SKT
                        while done < upto:
                            prev_pieces[done]()
                            done += 1
                        ps = psq.tile([P, psz], F32, tag="qk",
                                      name=f"{a}_qk_{h}_{t}_{poff}")
                        c0 = 0
                        for n in csizes:
                            nc.tensor.matmul(
                                ps[:, c0:c0 + n],
                                kt_sb[:, h // 2, P * t:P * (t + 1)],
                                qt_pad[:, h, poff + c0:poff + c0 + n],
                                start=True, stop=True)
                            c0 += n
                        nc.scalar.activation(out=at[:, t, :psz],
                                             in_=ps[:, :psz],
                                             func=Exp, scale=0.125)
                    while done < L:
                        prev_pieces[done]()
                        done += 1
                    prev_pieces = pv_pieces(h, at, poff, psz)
            for piece in prev_pieces:
                piece()


def _build_program(arm_specs, single_arm=None):
    """Build + bacc-compile the SPMD program for the given arm shapes.

    single_arm: if not None, build a plain (no Switch) program with just
    that arm -- used for CoreSim validation of one arm.
    """
    _ensure_paths()
    import concourse.bass as bass  # noqa: F401
    import concourse.tile as tile
    from concourse import bacc, mybir

    BF = mybir.dt.bfloat16
    F32 = mybir.dt.float32

    SQTmax = max(s[1] for s in arm_specs)
    SKTmax = max(s[2] for s in arm_specs)
    Emax = max(s[0] for s in arm_specs) * DH
    SQmax, SKmax = SQTmax * P, SKTmax * P

    nc = bacc.Bacc("TRN2", target_bir_lowering=False, debug=False,
                   num_devices=NCORES)

    xqt = nc.dram_tensor("xqt", [D, SQmax], BF, kind="ExternalInput").ap()
    xkt = nc.dram_tensor("xkt", [D, SKmax], BF, kind="ExternalInput").ap()
    xvt = nc.dram_tensor("xvt", [D, SKmax], BF, kind="ExternalInput").ap()
    wq = nc.dram_tensor("wq", [D, Emax], BF, kind="ExternalInput").ap()
    wk = nc.dram_tensor("wk", [D, Emax], BF, kind="ExternalInput").ap()
    wv = nc.dram_tensor("wv", [D, Emax], BF, kind="ExternalInput").ap()
    maskq = nc.dram_tensor("maskq", [SQTmax, P], F32, kind="ExternalInput").ap()
    maskk = nc.dram_tensor("maskk", [SKmax, H, 1], BF, kind="ExternalInput").ap()
    vid_t = nc.dram_tensor("vid", [1, 1], mybir.dt.uint32,
                           kind="ExternalInput")
    out = nc.dram_tensor("out", [SQmax, Emax], F32, kind="ExternalOutput").ap()
    tensors = (xqt, xkt, xvt, wq, wk, wv, maskq, maskk, out)

    n_arms = len(arm_specs)

    with tile.TileContext(nc) as tc:
        if single_arm is not None:
            nh, sqt, skt = arm_specs[single_arm]
            _emit_arm(nc, tc, tensors, single_arm, nh, sqt, skt)
        elif n_arms == 1:
            nh, sqt, skt = arm_specs[0]
            _emit_arm(nc, tc, tensors, 0, nh, sqt, skt)
        else:
            tmp = nc.alloc_registers(f"vid_{nc.next_id()}",
                                     mybir.ALL_ENGINES)
            nc.regs_load(tmp, vid_t.ap()[0:1, 0:1])
            vid = nc.snap(tmp, donate=True, min_val=0,
                          max_val=n_arms - 1)
            for case in tc.Switch(vid, n_arms):
                nh, sqt, skt = arm_specs[case]
                _emit_arm(nc, tc, tensors, case, nh, sqt, skt)

    nc.compile()
    return nc


def _get_program(arm_specs, single_arm=None):
    key = (arm_specs, single_arm)
    if key not in _PROG_CACHE:
        _PROG_CACHE[key] = _build_program(arm_specs, single_arm)
    return _PROG_CACHE[key]


def _prep_in_maps(Q_seq, K_seq, V_seq, WQ, WK, WV, Q_len, V_len,
                  arm_specs, assignment):
    BF = ml_dtypes.bfloat16
    SQTmax = max(s[1] for s in arm_specs)
    SKTmax = max(s[2] for s in arm_specs)
    Emax = max(s[0] for s in arm_specs) * DH
    SQmax, SKmax = SQTmax * P, SKTmax * P

    in_maps = []
    for c in range(NCORES):
        arm, b, hlo = assignment[c]
        NH, SQT, SKT = arm_specs[arm]
        E = NH * DH
        SQ, SK = SQT * P, SKT * P
        ql, vl = int(Q_len[b]), int(V_len[b])
        mk = (np.arange(SKmax) < vl)

        xq = np.zeros((D, SQmax), dtype=BF)
        xq[:, :SQ] = np.ascontiguousarray(Q_seq[b, :SQ].T).astype(BF)
        xk = np.zeros((D, SKmax), dtype=BF)
        xk[:, :SK] = np.ascontiguousarray(K_seq[b, :SK].T).astype(BF)
        xv = np.zeros((D, SKmax), dtype=BF)
        xv[:, :SK] = np.ascontiguousarray(
            (V_seq[b, :SK] * mk[:SK, None]).T).astype(BF)

        wqc = np.zeros((D, Emax), dtype=BF)
        wqc[:, :E] = WQ[:, DH * hlo:DH * (hlo + NH)].astype(BF)
        wkc = np.zeros((D, Emax), dtype=BF)
        wkc[:, :E] = WK[:, DH * hlo:DH * (hlo + NH)].astype(BF)
        wvc = np.zeros((D, Emax), dtype=BF)
        wvc[:, :E] = WV[:, DH * hlo:DH * (hlo + NH)].astype(BF)

        mq = np.zeros((SQTmax, P), dtype=np.float32)
        mq[:SQT] = (np.arange(SQ) < ql).astype(np.float32).reshape(SQT, P)

        in_maps.append({
            "xqt": xq, "xkt": xk, "xvt": xv,
            "wq": wqc, "wk": wkc, "wv": wvc,
            "maskq": mq,
            "maskk": np.repeat(mk.astype(BF)[:, None], H, axis=1)[..., None],
            "vid": np.array([[arm]], dtype=np.uint32),
        })
    return in_maps


def kernel(Q_seq, K_seq, V_seq, WQ, WK, WV, Q_len, V_len):
    global _last_nc, _last_in_maps
    _ensure_paths()
    from concourse.bass_utils import run_bass_kernel_spmd

    Q_seq = np.asarray(Q_seq, dtype=np.float32)
    K_seq = np.asarray(K_seq, dtype=np.float32)
    V_seq = np.asarray(V_seq, dtype=np.float32)
    WQ = np.asarray(WQ, dtype=np.float32)
    WK = np.asarray(WK, dtype=np.float32)
    WV = np.asarray(WV, dtype=np.float32)
    Q_len = np.asarray(Q_len).reshape(-1)
    V_len = np.asarray(V_len).reshape(-1)

    B, S, _ = Q_seq.shape
    arm_specs, assignment = _plan(Q_len, V_len)
    nc = _get_program(arm_specs)
    in_maps = _prep_in_maps(Q_seq, K_seq, V_seq, WQ, WK, WV, Q_len, V_len,
                            arm_specs, assignment)

    res = run_bass_kernel_spmd(nc, in_maps, core_ids=list(range(NCORES)))
    _last_nc, _last_in_maps = nc, in_maps

    full = np.zeros((B, S, H * DH), dtype=np.float32)
    for c in range(NCORES):
        arm, b, hlo = assignment[c]
        NH, SQT, SKT = arm_specs[arm]
        SQ = SQT * P
        o = res.results[c]["out"]
        full[b, :SQ, DH * hlo:DH * (hlo + NH)] = o[:SQ, :NH * DH]
    return full
